# revision 47
# baseline (speedup 1.0000x reference)
"""Trainium2 Bass kernel for pre-LN multi-head self-attention.

Module: y = LN(x); qkv = y @ w_qkv; attention(8 heads, dh=64); out = ao @ w_out
Shapes: x [4, 2048, 512], w_qkv [512, 1536], w_out [512, 512], fp32.

Sharding (8 cores): core c -> batch b = c//2, head-group g = c%2 (4 heads).
Each core computes LN + QKV (its head slice) + attention + a partial output
projection (its heads' rows of w_out); the host sums the two partials per batch.

Per-core dataflow:
  Warm-up burst of dummy matmuls un-throttles the PE HAM clock gate while the
  x DMA lands.  LN in natural [tok, d] layout (bn_stats) -> PE-transpose ->
  yT [d, tok]; per 512-token group the transposes are followed immediately by
  that group's K and V projections so the PE stream stays dense.  Q is
  projected lazily (block qb+1 while attention on block qb runs).
  Attention runs per (q-block 512, head-pair, k-block): the two heads' score
  matmuls have contraction dh=64, so they execute CONCURRENTLY on the two
  64-row PE tiles (T0/T8 row tiling); one ACT exp [128,1024] covers both
  heads.  attn@V accumulates [dh+1, 512] per head in PSUM, with a fused
  ones-column in V accumulating the softmax denominator in row 64.
  Normalization: reciprocal_approx_fast on the denominator row, GPSIMD
  partition_broadcast across the 64 feature partitions, one DVE multiply
  PSUM->SBUF producing bf16 aoT.  Output projection (2 matmuls per 128-token
  tile) interleaves into the attention stream one q-block behind.
ln_scale/ln_bias are folded into w_qkv on the host (w_eff = scale*W,
bias_row = bias@W added per-feature on device).  Matmul operands are bf16
(PSUM accumulation fp32).  In stage D the ACT engine does exp exclusively
(it is the throughput limiter at 1 elem/cycle/lane); all evictions and
normalization run on DVE/GPSIMD.
"""

import sys

if "/opt/trn_rl_repo" not in sys.path:
    sys.path.insert(0, "/opt/trn_rl_repo")

from contextlib import ExitStack

import numpy as np

import concourse.bass as bass
import concourse.tile as tile
from concourse.masks import make_identity
from concourse import bacc, mybir
from concourse.bass_utils import run_bass_kernel_spmd

B, N, D = 4, 2048, 512
H, DH = 8, 64
HPC = 4                 # heads per core
FPC = HPC * DH          # 256 features per core
P = 128
NT = N // P             # 16 token tiles
DT = D // P             # 4 d tiles
NG = N // 512           # 4 token groups / q blocks of 512
KB = NT                 # 16 k blocks of 128
EPS = 1e-6
SCALE = DH ** -0.5
F32 = mybir.dt.float32
BF16 = mybir.dt.bfloat16
ALU = mybir.AluOpType
AFT = mybir.ActivationFunctionType

import os
# debug switches (bisection): default fast path
FAST_RECIP = os.environ.get("K_FAST_RECIP", "1") == "1"
GPS_BCAST = os.environ.get("K_GPS_BCAST", "1") == "1"
PACK_SCORES = os.environ.get("K_PACK_SCORES", "1") == "1"
WARMUP = os.environ.get("K_WARMUP", "1") == "1"


def build_kernel():
    nc = bacc.Bacc("TRN2", target_bir_lowering=False, debug=False)
    xb = nc.dram_tensor("xb", [N, D], F32, kind="ExternalInput").ap()
    wq = nc.dram_tensor("wq", [D, FPC], BF16, kind="ExternalInput").ap()
    wk = nc.dram_tensor("wk", [D, FPC], BF16, kind="ExternalInput").ap()
    wv = nc.dram_tensor("wv", [D, FPC], BF16, kind="ExternalInput").ap()
    wo = nc.dram_tensor("wo", [FPC, D], BF16, kind="ExternalInput").ap()
    bq = nc.dram_tensor("bq", [FPC], F32, kind="ExternalInput").ap()
    bk = nc.dram_tensor("bk", [FPC], F32, kind="ExternalInput").ap()
    bv = nc.dram_tensor("bv", [FPC], F32, kind="ExternalInput").ap()
    out = nc.dram_tensor("out", [N, D], F32, kind="ExternalOutput").ap()

    with tile.TileContext(nc, pool_alloc_mode="queue") as tc, ExitStack() as ctx:
        consts = ctx.enter_context(tc.tile_pool(name="consts", bufs=1))
        big = ctx.enter_context(tc.tile_pool(name="big", bufs=1))
        dram = ctx.enter_context(tc.tile_pool(name="dram", bufs=2, space="DRAM"))

        # ---- PE warm-up first: one long accumulation group of dummy matmuls
        # (no per-MM semaphores) so the PE is continuously busy >3.4us and the
        # HAM clock gate flips to 8/8 before the real matmuls start ----
        junk = consts.tile([P, P], BF16)
        nc.vector.memset(junk, 0.0)
        if WARMUP:
            with tc.tile_pool(name="warm_ps", bufs=1, space="PSUM") as wpp:
                wt = wpp.tile([P, P], F32, tag="warm", name="warm")
                for w in range(36):
                    mm = nc.tensor.matmul(
                        wt, lhsT=junk, rhs=junk, start=(w == 0), stop=(w == 35)
                    )
                    if w > 0:
                        mm.ins.ldweights = False

        identity = consts.tile([P, P], BF16)
        make_identity(nc, identity)
        eps_t = consts.tile([P, 1], F32)
        nc.vector.memset(eps_t, EPS)

        yT = [big.tile([P, N], BF16, tag=f"yT{j}", name=f"yT{j}") for j in range(DT)]
        qT = [big.tile([P, N], BF16, tag=f"qT{j}", name=f"qT{j}") for j in range(2)]
        kT = [big.tile([P, N], BF16, tag=f"kT{j}", name=f"kT{j}") for j in range(2)]
        # per-(feature-half, q-block) tiles: block-local writes/reads so the
        # output projection of block qb never serializes against a later
        # block's normalize in the dependency tracker
        aoT = [
            [
                big.tile([P, 512], BF16, tag=f"aoT{j}_{qb}", name=f"aoT{j}_{qb}")
                for qb in range(NG)
            ]
            for j in range(2)
        ]
        v_sb = big.tile([P, NT, HPC, DH + 1], BF16)
        ones_col = consts.tile([P, 1], F32)
        nc.vector.memset(ones_col, 1.0)
        nc.vector.tensor_copy(
            v_sb[:, :, :, DH : DH + 1],
            ones_col[:, 0:1].to_broadcast((P, NT, HPC, 1)),
        )

        def kq_group(pool, ig, w_sb, b_sb, dstT, on_act, tag="kq", js=(0, 1)):
            # dstT[j][:, ig block] for j = 0, 1 (feature halves)
            for j in js:
                ps = pool.tile([P, 512], F32, tag=tag, name=f"kq{ig}_{j}_{on_act}")
                for dt in range(DT):
                    nc.tensor.matmul(
                        ps,
                        lhsT=(w_sb[:, dt, j * P : (j + 1) * P]),
                        rhs=(yT[dt][:, ig * 512 : (ig + 1) * 512]),
                        start=(dt == 0),
                        stop=(dt == DT - 1),
                    )
                if on_act:
                    nc.scalar.activation(
                        out=dstT[j][:, ig * 512 : (ig + 1) * 512],
                        in_=ps,
                        func=AFT.Identity,
                        bias=b_sb[:, j : j + 1],
                    )
                else:
                    nc.vector.tensor_scalar(
                        out=dstT[j][:, ig * 512 : (ig + 1) * 512],
                        in0=ps,
                        scalar1=b_sb[:, j : j + 1],
                        scalar2=None,
                        op0=ALU.add,
                    )

        # ---- Stages A-C: LayerNorm, transpose, K/V projections (per token
        # group so the PE alternates short transpose and matmul bursts), and
        # the Q projection for q-block 0.  Q for blocks 1-3 is issued inside
        # stage D one block ahead of use, filling PE slack in the exp-bound
        # attention loop. ----
        with tc.tile_pool(name="ln", bufs=6) as ln, tc.tile_pool(
            name="tp_psum", bufs=4, space="PSUM"
        ) as tpp, tc.tile_pool(name="c_psum", bufs=2, space="PSUM") as cpp, tc.tile_pool(
            name="v_psum", bufs=2, space="PSUM"
        ) as vpp:
            y_groups = []
            for ig in range(NG):
                y_ts = []
                for ii in range(4):
                    i = ig * 4 + ii
                    x_t = ln.tile([P, D], F32, tag="x")
                    nc.sync.dma_start(out=x_t, in_=xb[i * P : (i + 1) * P, :])
                    stats = ln.tile([P, 6], F32, tag="stats")
                    nc.vector.bn_stats(out=stats, in_=x_t)
                    mv = ln.tile([P, 2], F32, tag="mv")
                    nc.vector.bn_aggr(out=mv, in_=stats)
                    std = ln.tile([P, 1], F32, tag="std")
                    nc.scalar.activation(
                        out=std, in_=mv[:, 1:2], func=AFT.Sqrt, bias=eps_t[:, 0:1]
                    )
                    rstd = ln.tile([P, 1], F32, tag="rstd")
                    nc.vector.reciprocal(out=rstd, in_=std)
                    y_t = ln.tile([P, D], BF16, tag="y", bufs=16)
                    nc.vector.tensor_scalar(
                        out=y_t,
                        in0=x_t,
                        scalar1=mv[:, 0:1],
                        scalar2=rstd[:, 0:1],
                        op0=ALU.subtract,
                        op1=ALU.mult,
                    )
                    y_ts.append(y_t)
                y_groups.append(y_ts)

            # weights: [d, f] -> sbuf [p, dt, f] (issued after the x DMAs so
            # the first LayerNorm group isn't stuck behind them in the queue)
            w_q_sb = consts.tile([P, DT, FPC], BF16)
            nc.sync.dma_start(out=w_q_sb, in_=wq.rearrange("(t p) f -> p t f", p=P))
            w_k_sb = consts.tile([P, DT, FPC], BF16)
            nc.sync.dma_start(out=w_k_sb, in_=wk.rearrange("(t p) f -> p t f", p=P))
            w_v_sb = consts.tile([P, DT, FPC], BF16)
            nc.sync.dma_start(out=w_v_sb, in_=wv.rearrange("(t p) f -> p t f", p=P))
            w_o_sb = consts.tile([P, 2, D], BF16)
            nc.sync.dma_start(out=w_o_sb, in_=wo.rearrange("(t p) f -> p t f", p=P))
            bq_sb = consts.tile([P, 2], F32)
            nc.sync.dma_start(out=bq_sb, in_=bq.rearrange("(t p) -> p t", p=P))
            bk_sb = consts.tile([P, 2], F32)
            nc.sync.dma_start(out=bk_sb, in_=bk.rearrange("(t p) -> p t", p=P))
            bv_b = consts.tile([P, FPC], F32)
            bv_bcast = bass.AP(
                tensor=bv.tensor, offset=bv.offset, ap=[[0, P]] + list(bv.ap)
            )
            nc.sync.dma_start(out=bv_b, in_=bv_bcast)

            def tp_group(ig):
                y_ts = y_groups[ig]
                for j in range(DT):
                    pt = tpp.tile([P, 512], BF16, tag="tp")
                    for ii in range(4):
                        nc.tensor.transpose(
                            pt[:, ii * P : (ii + 1) * P],
                            y_ts[ii][:, j * P : (j + 1) * P],
                            identity,
                        )
                    nc.scalar.activation(
                        out=yT[j][:, ig * 512 : (ig + 1) * 512],
                        in_=pt,
                        func=AFT.Copy,
                    )

            def v_group(ig):
                for ii in range(4):
                    i = ig * 4 + ii
                    ps = vpp.tile([P, FPC], F32, tag="v", name=f"v{i}")
                    for dt in range(DT):
                        nc.tensor.matmul(
                            ps,
                            lhsT=(yT[dt][:, i * P : (i + 1) * P]),
                            rhs=(w_v_sb[:, dt, :]),
                            start=(dt == 0),
                            stop=(dt == DT - 1),
                        )
                    nc.vector.tensor_tensor(
                        out=v_sb[:, i, :, 0:DH],
                        in0=ps.rearrange("p (h d) -> p h d", h=HPC),
                        in1=bv_b.rearrange("p (h d) -> p h d", h=HPC),
                        op=ALU.add,
                    )

            for ig in range(NG):
                tp_group(ig)
                kq_group(cpp, ig, w_k_sb, bk_sb, kT, on_act=True)
                v_group(ig)
            kq_group(cpp, 0, w_q_sb, bq_sb, qT, on_act=True)

        # ---- Stage D: attention, units of (q-block, head-pair, k-block).
        # Two heads' score matmuls run concurrently on the 64-row PE tiles
        # (contraction dh=64 -> row tiling T0/T8); one exp covers both. ----
        with tc.tile_pool(name="sc_psum", bufs=2, space="PSUM") as scp, tc.tile_pool(
            name="ao_psum", bufs=2, space="PSUM"
        ) as aop, tc.tile_pool(name="exp_sb", bufs=6) as exps, tc.tile_pool(
            name="nrm", bufs=4
        ) as nrm, tc.tile_pool(
            name="o_psum", bufs=2, space="PSUM"
        ) as opp, tc.tile_pool(name="o_sb", bufs=3) as osb:
            items = [
                (qb, hp, kb)
                for qb in range(NG)
                for hp in range(2)
                for kb in range(KB)
            ]
            ex_tiles = {}
            ao_tiles = {}

            def sc_exp(i):
                qb, hp, kb = items[i]
                q0 = qb * 512
                sc = scp.tile([P, 1024], F32, tag="sc", name=f"sc{i}")
                if PACK_SCORES:
                    # concurrent 64-row PE tiles: head 2hp on rows 0-63 (T0),
                    # head 2hp+1 on rows 64-127 (T8); different PSUM banks
                    nc.tensor.matmul(
                        sc[:, 0:512],
                        lhsT=(kT[hp][0:DH, kb * P : (kb + 1) * P]),
                        rhs=(qT[hp][0:DH, q0 : q0 + 512]),
                        start=True,
                        stop=True,
                    )
                    nc.tensor.matmul(
                        sc[:, 512:1024],
                        lhsT=(kT[hp][DH : 2 * DH, kb * P : (kb + 1) * P]),
                        rhs=(qT[hp][DH : 2 * DH, q0 : q0 + 512]),
                        start=True,
                        stop=True,
                    )
                else:
                    for h2 in range(2):
                        nc.tensor.matmul(
                            sc[:, h2 * 512 : (h2 + 1) * 512],
                            lhsT=(
                                kT[hp][h2 * DH : (h2 + 1) * DH, kb * P : (kb + 1) * P]
                            ),
                            rhs=(qT[hp][h2 * DH : (h2 + 1) * DH, q0 : q0 + 512]),
                            start=True,
                            stop=True,
                        )
                ex = exps.tile([P, 1024], BF16, tag="ex", name=f"ex{i}")
                nc.scalar.activation(out=ex, in_=sc, func=AFT.Exp, scale=SCALE)
                ex_tiles[i] = ex

            def evict_ao(qb, hp, h2, ao_ps):
                # One [65,512] copy PSUM->SBUF so the PSUM bank frees quickly
                # (the next pair's attn@V reuses it ~1.1us later).
                ao_sb = nrm.tile(
                    [DH + 1, 512], F32, tag="ao_sb", name=f"as{qb}_{hp}_{h2}", bufs=3
                )
                nc.vector.tensor_copy(ao_sb, ao_ps)
                return ao_sb

            def norm_prep(qb, hp, h2, ao_sb, cs, cw):
                # reciprocal of the denominator row, broadcast over 64
                # partitions; issued for BOTH heads before any multiply so
                # the DVE never idles waiting on a GPSIMD broadcast
                sfx = f"{qb}_{hp}_{h2}_{cs}"
                # reciprocal_approx_fast (custom DVE op) requires its input at
                # partition 0 -> re-copy the denominator row SBUF->SBUF
                den = nrm.tile([1, cw], F32, tag="den", name=f"dn{sfx}")
                nc.vector.tensor_copy(den, ao_sb[DH : DH + 1, cs : cs + cw])
                r = nrm.tile([1, cw], F32, tag="r", name=f"r{sfx}")
                if FAST_RECIP:
                    nc.vector.reciprocal_approx_fast(out=r, in_=den)
                else:
                    nc.vector.reciprocal(out=r, in_=den)
                rb = nrm.tile([DH, cw], F32, tag="rb", name=f"rb{sfx}")
                # h2=0 broadcasts on GPSIMD, h2=1 via a DMA round-trip: the
                # two run on independent engines instead of serializing on
                # GPSIMD, shortening the normalize chain's critical path
                if GPS_BCAST and h2 == 0:
                    nc.gpsimd.partition_broadcast(rb, r[0:1, :], channels=DH)
                else:
                    scr = dram.tile([1, cw], F32, tag="scr", name=f"scr{sfx}")
                    nc.sync.dma_start(out=scr, in_=r)
                    nc.sync.dma_start(out=rb, in_=scr[0:1, :].to_broadcast((DH, cw)))
                return rb

            def norm_apply(qb, hp, h2, ao_sb, rb, cs, cw):
                nc.vector.tensor_tensor(
                    out=aoT[hp][qb][h2 * DH : (h2 + 1) * DH, cs : cs + cw],
                    in0=ao_sb[0:DH, cs : cs + cw],
                    in1=rb,
                    op=ALU.mult,
                )

            def oproj_tile(mt, on_act=False):
                ps = opp.tile([P, D], F32, tag="o", name=f"o{mt}")
                for kt in range(2):
                    nc.tensor.matmul(
                        ps,
                        lhsT=(aoT[kt][mt // 4][:, (mt % 4) * P : (mt % 4 + 1) * P]),
                        rhs=(w_o_sb[:, kt, :]),
                        start=(kt == 0),
                        stop=(kt == 1),
                    )
                ot = osb.tile([P, D], F32, tag="ot", name=f"ot{mt}")
                if on_act:
                    # tail: exps are done, ACT is idle -> evict there so the
                    # DVE normalize chain and the oproj chain run in parallel
                    nc.scalar.activation(out=ot, in_=ps, func=AFT.Copy)
                else:
                    nc.vector.tensor_copy(ot, ps)
                nc.sync.dma_start(out=out[mt * P : (mt + 1) * P, :], in_=ot)

            def attn_v(i):
                qb, hp, kb = items[i]
                if kb < 2:
                    # defer kb=0/1 two units: the fresh PSUM accumulators
                    # reuse banks the previous pair's evict copies are still
                    # vacating; the extra slack absorbs DVE queue jitter
                    return
                if kb == 2:
                    for h2 in range(2):
                        ao_tiles[(qb, hp, h2)] = aop.tile(
                            [DH + 1, 512], F32, tag="ao", name=f"ao{qb}_{hp}_{h2}"
                        )
                kbs = [0, 1, 2] if kb == 2 else [kb]
                for kbx in kbs:
                    ex = ex_tiles.pop(i - kb + kbx)
                    for h2 in range(2):
                        nc.tensor.matmul(
                            ao_tiles[(qb, hp, h2)],
                            lhsT=(v_sb[:, kbx, 2 * hp + h2, :]),
                            rhs=(ex[:, h2 * 512 : (h2 + 1) * 512]),
                            start=(kbx == 0),
                            stop=(kbx == KB - 1),
                        )
                if kb == KB - 1:
                    last = qb == NG - 1 and hp == 1
                    ao_sbs = [
                        evict_ao(qb, hp, h2, ao_tiles.pop((qb, hp, h2)))
                        for h2 in range(2)
                    ]
                    if not last:
                        # 2 chunks of 256: the first chunk's multiplies land
                        # ~4us earlier, so the oproj LDWEIGHTS reading this
                        # block's aoT never stalls the PE head-of-line
                        for cs in (0, 256):
                            rbs = [
                                norm_prep(qb, hp, h2, ao_sbs[h2], cs, 256)
                                for h2 in range(2)
                            ]
                            for h2 in range(2):
                                norm_apply(
                                    qb, hp, h2, ao_sbs[h2], rbs[h2], cs, 256
                                )
                    else:
                        # final pair: interleave per-128-token normalize chunks
                        # with the output projection so the tail pipelines
                        for ch in range(4):
                            rbs = [
                                norm_prep(qb, hp, h2, ao_sbs[h2], ch * P, P)
                                for h2 in range(2)
                            ]
                            for h2 in range(2):
                                norm_apply(qb, hp, h2, ao_sbs[h2], rbs[h2], ch * P, P)
                            oproj_tile(qb * 4 + ch, on_act=True)

            DEPTH = 2
            for i in range(min(DEPTH, len(items))):
                sc_exp(i)
            for i in range(len(items)):
                if i + DEPTH < len(items):
                    sc_exp(i + DEPTH)
                attn_v(i)
                qb, hp, kb = items[i]
                # lazy Q projection: block qb+1 while attention runs on qb,
                # one feature half per unit (the PE has only ~200ns/unit of
                # slack in the exp-bound loop -- bursts stall the pipeline)
                # oproj(qb-1) is injected a full head-pair sweep after the
                # (qb-1, hp=1) normalize chain (~6us of DVE+GPSIMD): the Tile
                # scheduler hoists injected work ~7 units earlier than issue
                # position, so the margin must be structural, not positional
                if hp == 1 and kb in (2, 4, 6, 8) and qb >= 1:
                    oproj_tile((qb - 1) * 4 + (kb - 2) // 2)
                if hp == 1 and kb in (10, 12) and qb + 1 < NG:
                    kq_group(
                        opp, qb + 1, w_q_sb, bq_sb, qT,
                        on_act=False, tag="o", js=(0 if kb == 10 else 1,),
                    )

    nc.compile()
    return nc


_NC_CACHE = None
_LAST_RESULT = None


def kernel(x, ln_scale, ln_bias, w_qkv, w_out):
    global _NC_CACHE, _LAST_RESULT
    if _NC_CACHE is None:
        _NC_CACHE = build_kernel()
    nc = _NC_CACHE

    import ml_dtypes

    x = np.asarray(x, np.float32)
    w_eff = (np.asarray(ln_scale, np.float32)[:, None] * np.asarray(w_qkv, np.float32))
    b_row = np.asarray(ln_bias, np.float32) @ np.asarray(w_qkv, np.float32)
    w_eff = w_eff.astype(ml_dtypes.bfloat16)
    w_out = np.asarray(w_out, np.float32).astype(ml_dtypes.bfloat16)

    in_maps = []
    for c in range(8):
        b, g = c // 2, c % 2
        s = slice(FPC * g, FPC * g + FPC)
        ks = slice(512 + FPC * g, 512 + FPC * g + FPC)
        vs = slice(1024 + FPC * g, 1024 + FPC * g + FPC)
        in_maps.append(
            {
                "xb": np.ascontiguousarray(x[b]),
                "wq": np.ascontiguousarray(w_eff[:, s]),
                "wk": np.ascontiguousarray(w_eff[:, ks]),
                "wv": np.ascontiguousarray(w_eff[:, vs]),
                "wo": np.ascontiguousarray(w_out[s, :]),
                "bq": np.ascontiguousarray(b_row[s]),
                "bk": np.ascontiguousarray(b_row[ks]),
                "bv": np.ascontiguousarray(b_row[vs]),
            }
        )
    res = run_bass_kernel_spmd(nc, in_maps, core_ids=list(range(8)))
    _LAST_RESULT = res
    outs = [res.results[c]["out"] for c in range(8)]
    return np.stack([outs[2 * b] + outs[2 * b + 1] for b in range(B)]).astype(
        np.float32
    )


if __name__ == "__main__":
    xs = np.random.randn(B, N, D).astype(np.float32)
    o = kernel(
        x=xs,
        ln_scale=np.ones(D, np.float32),
        ln_bias=np.zeros(D, np.float32),
        w_qkv=(np.random.randn(D, 3 * H * DH) / np.sqrt(D)).astype(np.float32),
        w_out=(np.random.randn(H * DH, D) / np.sqrt(H * DH)).astype(np.float32),
    )
    print(o.shape, o.dtype)


# revision 48
# speedup vs baseline: 1.0358x; 1.0358x over previous
"""Trainium2 Bass kernel for pre-LN multi-head self-attention.

Module: y = LN(x); qkv = y @ w_qkv; attention(8 heads, dh=64); out = ao @ w_out
Shapes: x [4, 2048, 512], w_qkv [512, 1536], w_out [512, 512], fp32.

Sharding (8 cores): core c -> batch b = c//2, head-group g = c%2 (4 heads).
Each core computes LN + QKV (its head slice) + attention + a partial output
projection (its heads' rows of w_out); the host sums the two partials per batch.

Per-core dataflow:
  Warm-up burst of dummy matmuls un-throttles the PE HAM clock gate while the
  x DMA lands.  LN in natural [tok, d] layout (bn_stats) -> PE-transpose ->
  yT [d, tok]; per 512-token group the transposes are followed immediately by
  that group's K and V projections so the PE stream stays dense.  Q is
  projected lazily (block qb+1 while attention on block qb runs).
  Attention runs per (q-block 512, head-pair, k-block): the two heads' score
  matmuls have contraction dh=64, so they execute CONCURRENTLY on the two
  64-row PE tiles (T0/T8 row tiling); one ACT exp [128,1024] covers both
  heads.  attn@V accumulates [dh+1, 512] per head in PSUM, with a fused
  ones-column in V accumulating the softmax denominator in row 64.
  Normalization: reciprocal_approx_fast on the denominator row, GPSIMD
  partition_broadcast across the 64 feature partitions, one DVE multiply
  PSUM->SBUF producing bf16 aoT.  Output projection (2 matmuls per 128-token
  tile) interleaves into the attention stream one q-block behind.
ln_scale/ln_bias are folded into w_qkv on the host (w_eff = scale*W,
bias_row = bias@W added per-feature on device).  Matmul operands are bf16
(PSUM accumulation fp32).  In stage D the ACT engine does exp exclusively
(it is the throughput limiter at 1 elem/cycle/lane); all evictions and
normalization run on DVE/GPSIMD.
"""

import sys

if "/opt/trn_rl_repo" not in sys.path:
    sys.path.insert(0, "/opt/trn_rl_repo")

from contextlib import ExitStack

import numpy as np

import concourse.bass as bass
import concourse.tile as tile
from concourse.masks import make_identity
from concourse import bacc, mybir
from concourse.bass_utils import run_bass_kernel_spmd

B, N, D = 4, 2048, 512
H, DH = 8, 64
HPC = 4                 # heads per core
FPC = HPC * DH          # 256 features per core
P = 128
NT = N // P             # 16 token tiles
DT = D // P             # 4 d tiles
NG = N // 512           # 4 token groups / q blocks of 512
KB = NT                 # 16 k blocks of 128
EPS = 1e-6
SCALE = DH ** -0.5
F32 = mybir.dt.float32
BF16 = mybir.dt.bfloat16
ALU = mybir.AluOpType
AFT = mybir.ActivationFunctionType

import os
# debug switches (bisection): default fast path
FAST_RECIP = os.environ.get("K_FAST_RECIP", "1") == "1"
GPS_BCAST = os.environ.get("K_GPS_BCAST", "1") == "1"
PACK_SCORES = os.environ.get("K_PACK_SCORES", "1") == "1"
WARMUP = os.environ.get("K_WARMUP", "1") == "1"


def build_kernel():
    nc = bacc.Bacc("TRN2", target_bir_lowering=False, debug=False)
    xb = nc.dram_tensor("xb", [N, D], F32, kind="ExternalInput").ap()
    wq = nc.dram_tensor("wq", [D, FPC], BF16, kind="ExternalInput").ap()
    wk = nc.dram_tensor("wk", [D, FPC], BF16, kind="ExternalInput").ap()
    wv = nc.dram_tensor("wv", [D, FPC], BF16, kind="ExternalInput").ap()
    wo = nc.dram_tensor("wo", [FPC, D], BF16, kind="ExternalInput").ap()
    bq = nc.dram_tensor("bq", [FPC], F32, kind="ExternalInput").ap()
    bk = nc.dram_tensor("bk", [FPC], F32, kind="ExternalInput").ap()
    bv = nc.dram_tensor("bv", [FPC], F32, kind="ExternalInput").ap()
    out = nc.dram_tensor("out", [N, D], F32, kind="ExternalOutput").ap()

    with tile.TileContext(nc, pool_alloc_mode="queue") as tc, ExitStack() as ctx:
        consts = ctx.enter_context(tc.tile_pool(name="consts", bufs=1))
        big = ctx.enter_context(tc.tile_pool(name="big", bufs=1))
        dram = ctx.enter_context(tc.tile_pool(name="dram", bufs=2, space="DRAM"))

        # ---- PE warm-up first: one long accumulation group of dummy matmuls
        # (no per-MM semaphores) so the PE is continuously busy >3.4us and the
        # HAM clock gate flips to 8/8 before the real matmuls start ----
        junk = consts.tile([P, P], BF16)
        nc.vector.memset(junk, 0.0)
        if WARMUP:
            with tc.tile_pool(name="warm_ps", bufs=1, space="PSUM") as wpp:
                wt = wpp.tile([P, P], F32, tag="warm", name="warm")
                for w in range(36):
                    mm = nc.tensor.matmul(
                        wt, lhsT=junk, rhs=junk, start=(w == 0), stop=(w == 35)
                    )
                    if w > 0:
                        mm.ins.ldweights = False

        identity = consts.tile([P, P], BF16)
        make_identity(nc, identity)
        eps_t = consts.tile([P, 1], F32)
        nc.vector.memset(eps_t, EPS)

        yT = [big.tile([P, N], BF16, tag=f"yT{j}", name=f"yT{j}") for j in range(DT)]
        qT = [big.tile([P, N], BF16, tag=f"qT{j}", name=f"qT{j}") for j in range(2)]
        kT = [big.tile([P, N], BF16, tag=f"kT{j}", name=f"kT{j}") for j in range(2)]
        # per-(feature-half, q-block) tiles: block-local writes/reads so the
        # output projection of block qb never serializes against a later
        # block's normalize in the dependency tracker
        aoT = [
            [
                big.tile([P, 512], BF16, tag=f"aoT{j}_{qb}", name=f"aoT{j}_{qb}")
                for qb in range(NG)
            ]
            for j in range(2)
        ]
        v_sb = big.tile([P, NT, HPC, DH + 1], BF16)
        ones_col = consts.tile([P, 1], F32)
        nc.vector.memset(ones_col, 1.0)
        nc.vector.tensor_copy(
            v_sb[:, :, :, DH : DH + 1],
            ones_col[:, 0:1].to_broadcast((P, NT, HPC, 1)),
        )

        def kq_group(pool, ig, w_sb, b_sb, dstT, on_act, tag="kq", js=(0, 1)):
            # dstT[j][:, ig block] for j = 0, 1 (feature halves)
            for j in js:
                ps = pool.tile([P, 512], F32, tag=tag, name=f"kq{ig}_{j}_{on_act}")
                for dt in range(DT):
                    nc.tensor.matmul(
                        ps,
                        lhsT=(w_sb[:, dt, j * P : (j + 1) * P]),
                        rhs=(yT[dt][:, ig * 512 : (ig + 1) * 512]),
                        start=(dt == 0),
                        stop=(dt == DT - 1),
                    )
                if on_act:
                    nc.scalar.activation(
                        out=dstT[j][:, ig * 512 : (ig + 1) * 512],
                        in_=ps,
                        func=AFT.Identity,
                        bias=b_sb[:, j : j + 1],
                    )
                else:
                    nc.vector.tensor_scalar(
                        out=dstT[j][:, ig * 512 : (ig + 1) * 512],
                        in0=ps,
                        scalar1=b_sb[:, j : j + 1],
                        scalar2=None,
                        op0=ALU.add,
                    )

        # ---- Stages A-C: LayerNorm, transpose, K/V projections (per token
        # group so the PE alternates short transpose and matmul bursts), and
        # the Q projection for q-block 0.  Q for blocks 1-3 is issued inside
        # stage D one block ahead of use, filling PE slack in the exp-bound
        # attention loop. ----
        with tc.tile_pool(name="ln", bufs=6) as ln, tc.tile_pool(
            name="tp_psum", bufs=4, space="PSUM"
        ) as tpp, tc.tile_pool(name="c_psum", bufs=2, space="PSUM") as cpp, tc.tile_pool(
            name="v_psum", bufs=2, space="PSUM"
        ) as vpp:
            y_groups = []
            for ig in range(NG):
                y_ts = []
                for ii in range(4):
                    i = ig * 4 + ii
                    x_t = ln.tile([P, D], F32, tag="x")
                    nc.sync.dma_start(out=x_t, in_=xb[i * P : (i + 1) * P, :])
                    stats = ln.tile([P, 6], F32, tag="stats")
                    nc.vector.bn_stats(out=stats, in_=x_t)
                    mv = ln.tile([P, 2], F32, tag="mv")
                    nc.vector.bn_aggr(out=mv, in_=stats)
                    std = ln.tile([P, 1], F32, tag="std")
                    nc.scalar.activation(
                        out=std, in_=mv[:, 1:2], func=AFT.Sqrt, bias=eps_t[:, 0:1]
                    )
                    rstd = ln.tile([P, 1], F32, tag="rstd")
                    nc.vector.reciprocal(out=rstd, in_=std)
                    y_t = ln.tile([P, D], BF16, tag="y", bufs=16)
                    nc.vector.tensor_scalar(
                        out=y_t,
                        in0=x_t,
                        scalar1=mv[:, 0:1],
                        scalar2=rstd[:, 0:1],
                        op0=ALU.subtract,
                        op1=ALU.mult,
                    )
                    y_ts.append(y_t)
                y_groups.append(y_ts)

            # weights: [d, f] -> sbuf [p, dt, f] (issued after the x DMAs so
            # the first LayerNorm group isn't stuck behind them in the queue)
            w_q_sb = consts.tile([P, DT, FPC], BF16)
            nc.sync.dma_start(out=w_q_sb, in_=wq.rearrange("(t p) f -> p t f", p=P))
            w_k_sb = consts.tile([P, DT, FPC], BF16)
            nc.sync.dma_start(out=w_k_sb, in_=wk.rearrange("(t p) f -> p t f", p=P))
            w_v_sb = consts.tile([P, DT, FPC], BF16)
            nc.sync.dma_start(out=w_v_sb, in_=wv.rearrange("(t p) f -> p t f", p=P))
            w_o_sb = consts.tile([P, 2, D], BF16)
            nc.sync.dma_start(out=w_o_sb, in_=wo.rearrange("(t p) f -> p t f", p=P))
            bq_sb = consts.tile([P, 2], F32)
            nc.sync.dma_start(out=bq_sb, in_=bq.rearrange("(t p) -> p t", p=P))
            bk_sb = consts.tile([P, 2], F32)
            nc.sync.dma_start(out=bk_sb, in_=bk.rearrange("(t p) -> p t", p=P))
            bv_b = consts.tile([P, FPC], F32)
            bv_bcast = bass.AP(
                tensor=bv.tensor, offset=bv.offset, ap=[[0, P]] + list(bv.ap)
            )
            nc.sync.dma_start(out=bv_b, in_=bv_bcast)

            def tp_group(ig):
                y_ts = y_groups[ig]
                for j in range(DT):
                    pt = tpp.tile([P, 512], BF16, tag="tp")
                    for ii in range(4):
                        nc.tensor.transpose(
                            pt[:, ii * P : (ii + 1) * P],
                            y_ts[ii][:, j * P : (j + 1) * P],
                            identity,
                        )
                    nc.scalar.activation(
                        out=yT[j][:, ig * 512 : (ig + 1) * 512],
                        in_=pt,
                        func=AFT.Copy,
                    )

            def v_group(ig):
                for ii in range(4):
                    i = ig * 4 + ii
                    ps = vpp.tile([P, FPC], F32, tag="v", name=f"v{i}")
                    for dt in range(DT):
                        nc.tensor.matmul(
                            ps,
                            lhsT=(yT[dt][:, i * P : (i + 1) * P]),
                            rhs=(w_v_sb[:, dt, :]),
                            start=(dt == 0),
                            stop=(dt == DT - 1),
                        )
                    nc.vector.tensor_tensor(
                        out=v_sb[:, i, :, 0:DH],
                        in0=ps.rearrange("p (h d) -> p h d", h=HPC),
                        in1=bv_b.rearrange("p (h d) -> p h d", h=HPC),
                        op=ALU.add,
                    )

            for ig in range(NG):
                tp_group(ig)
                kq_group(cpp, ig, w_k_sb, bk_sb, kT, on_act=True)
                v_group(ig)
            kq_group(cpp, 0, w_q_sb, bq_sb, qT, on_act=True)

        # ---- Stage D: attention, units of (q-block, head-pair, k-block).
        # Two heads' score matmuls run concurrently on the 64-row PE tiles
        # (contraction dh=64 -> row tiling T0/T8); one exp covers both. ----
        with tc.tile_pool(name="sc_psum", bufs=2, space="PSUM") as scp, tc.tile_pool(
            name="ao_psum", bufs=2, space="PSUM"
        ) as aop, tc.tile_pool(name="exp_sb", bufs=6) as exps, tc.tile_pool(
            name="nrm", bufs=4
        ) as nrm, tc.tile_pool(
            name="o_psum", bufs=2, space="PSUM"
        ) as opp, tc.tile_pool(name="o_sb", bufs=3) as osb:
            items = [
                (qb, hp, kb)
                for qb in range(NG)
                for hp in range(2)
                for kb in range(KB)
            ]
            ex_tiles = {}
            ao_tiles = {}

            def sc_exp(i):
                qb, hp, kb = items[i]
                q0 = qb * 512
                sc = scp.tile([P, 1024], F32, tag="sc", name=f"sc{i}")
                if PACK_SCORES:
                    # concurrent 64-row PE tiles: head 2hp on rows 0-63 (T0),
                    # head 2hp+1 on rows 64-127 (T8); different PSUM banks
                    nc.tensor.matmul(
                        sc[:, 0:512],
                        lhsT=(kT[hp][0:DH, kb * P : (kb + 1) * P]),
                        rhs=(qT[hp][0:DH, q0 : q0 + 512]),
                        start=True,
                        stop=True,
                    )
                    nc.tensor.matmul(
                        sc[:, 512:1024],
                        lhsT=(kT[hp][DH : 2 * DH, kb * P : (kb + 1) * P]),
                        rhs=(qT[hp][DH : 2 * DH, q0 : q0 + 512]),
                        start=True,
                        stop=True,
                    )
                else:
                    for h2 in range(2):
                        nc.tensor.matmul(
                            sc[:, h2 * 512 : (h2 + 1) * 512],
                            lhsT=(
                                kT[hp][h2 * DH : (h2 + 1) * DH, kb * P : (kb + 1) * P]
                            ),
                            rhs=(qT[hp][h2 * DH : (h2 + 1) * DH, q0 : q0 + 512]),
                            start=True,
                            stop=True,
                        )
                ex = exps.tile([P, 1024], BF16, tag="ex", name=f"ex{i}")
                nc.scalar.activation(out=ex, in_=sc, func=AFT.Exp, scale=SCALE)
                ex_tiles[i] = ex

            def evict_ao(qb, hp, h2, ao_ps):
                # One [65,512] copy PSUM->SBUF so the PSUM bank frees quickly
                # (the next pair's attn@V reuses it ~1.1us later).
                ao_sb = nrm.tile(
                    [DH + 1, 512], F32, tag="ao_sb", name=f"as{qb}_{hp}_{h2}", bufs=3
                )
                nc.vector.tensor_copy(ao_sb, ao_ps)
                return ao_sb

            def norm_prep(qb, hp, h2, ao_sb, cs, cw):
                # reciprocal of the denominator row, broadcast over 64
                # partitions; issued for BOTH heads before any multiply so
                # the DVE never idles waiting on a GPSIMD broadcast
                sfx = f"{qb}_{hp}_{h2}_{cs}"
                # reciprocal_approx_fast (custom DVE op) requires its input at
                # partition 0 -> re-copy the denominator row SBUF->SBUF
                den = nrm.tile([1, cw], F32, tag="den", name=f"dn{sfx}")
                nc.vector.tensor_copy(den, ao_sb[DH : DH + 1, cs : cs + cw])
                r = nrm.tile([1, cw], F32, tag="r", name=f"r{sfx}")
                if FAST_RECIP:
                    nc.vector.reciprocal_approx_fast(out=r, in_=den)
                else:
                    nc.vector.reciprocal(out=r, in_=den)
                rb = nrm.tile([DH, cw], F32, tag="rb", name=f"rb{sfx}")
                if GPS_BCAST:
                    nc.gpsimd.partition_broadcast(rb, r[0:1, :], channels=DH)
                else:
                    scr = dram.tile([1, cw], F32, tag="scr", name=f"scr{sfx}")
                    nc.sync.dma_start(out=scr, in_=r)
                    nc.sync.dma_start(out=rb, in_=scr[0:1, :].to_broadcast((DH, cw)))
                return rb

            def norm_apply(qb, hp, h2, ao_sb, rb, cs, cw):
                nc.vector.tensor_tensor(
                    out=aoT[hp][qb][h2 * DH : (h2 + 1) * DH, cs : cs + cw],
                    in0=ao_sb[0:DH, cs : cs + cw],
                    in1=rb,
                    op=ALU.mult,
                )

            def oproj_tile(mt, on_act=False):
                ps = opp.tile([P, D], F32, tag="o", name=f"o{mt}")
                for kt in range(2):
                    nc.tensor.matmul(
                        ps,
                        lhsT=(aoT[kt][mt // 4][:, (mt % 4) * P : (mt % 4 + 1) * P]),
                        rhs=(w_o_sb[:, kt, :]),
                        start=(kt == 0),
                        stop=(kt == 1),
                    )
                ot = osb.tile([P, D], F32, tag="ot", name=f"ot{mt}")
                if on_act:
                    # tail: exps are done, ACT is idle -> evict there so the
                    # DVE normalize chain and the oproj chain run in parallel
                    nc.scalar.activation(out=ot, in_=ps, func=AFT.Copy)
                else:
                    nc.vector.tensor_copy(ot, ps)
                nc.sync.dma_start(out=out[mt * P : (mt + 1) * P, :], in_=ot)

            def attn_v(i):
                qb, hp, kb = items[i]
                if kb < 2:
                    # defer kb=0/1 two units: the fresh PSUM accumulators
                    # reuse banks the previous pair's evict copies are still
                    # vacating; the extra slack absorbs DVE queue jitter
                    return
                if kb == 2:
                    for h2 in range(2):
                        ao_tiles[(qb, hp, h2)] = aop.tile(
                            [DH + 1, 512], F32, tag="ao", name=f"ao{qb}_{hp}_{h2}"
                        )
                kbs = [0, 1, 2] if kb == 2 else [kb]
                for kbx in kbs:
                    ex = ex_tiles.pop(i - kb + kbx)
                    for h2 in range(2):
                        nc.tensor.matmul(
                            ao_tiles[(qb, hp, h2)],
                            lhsT=(v_sb[:, kbx, 2 * hp + h2, :]),
                            rhs=(ex[:, h2 * 512 : (h2 + 1) * 512]),
                            start=(kbx == 0),
                            stop=(kbx == KB - 1),
                        )
                if kb == KB - 1:
                    last = qb == NG - 1 and hp == 1
                    ao_sbs = [
                        evict_ao(qb, hp, h2, ao_tiles.pop((qb, hp, h2)))
                        for h2 in range(2)
                    ]
                    if not last:
                        # 2 chunks of 256: the first chunk's multiplies land
                        # ~4us earlier, so the oproj LDWEIGHTS reading this
                        # block's aoT never stalls the PE head-of-line
                        for cs in (0, 256):
                            rbs = [
                                norm_prep(qb, hp, h2, ao_sbs[h2], cs, 256)
                                for h2 in range(2)
                            ]
                            for h2 in range(2):
                                norm_apply(
                                    qb, hp, h2, ao_sbs[h2], rbs[h2], cs, 256
                                )
                    else:
                        # final pair: interleave per-128-token normalize chunks
                        # with the output projection so the tail pipelines
                        for ch in range(4):
                            rbs = [
                                norm_prep(qb, hp, h2, ao_sbs[h2], ch * P, P)
                                for h2 in range(2)
                            ]
                            for h2 in range(2):
                                norm_apply(qb, hp, h2, ao_sbs[h2], rbs[h2], ch * P, P)
                            oproj_tile(qb * 4 + ch, on_act=True)

            DEPTH = 2
            for i in range(min(DEPTH, len(items))):
                sc_exp(i)
            for i in range(len(items)):
                if i + DEPTH < len(items):
                    sc_exp(i + DEPTH)
                attn_v(i)
                qb, hp, kb = items[i]
                # lazy Q projection: block qb+1 while attention runs on qb,
                # one feature half per unit (the PE has only ~200ns/unit of
                # slack in the exp-bound loop -- bursts stall the pipeline)
                # oproj(qb-1) is injected a full head-pair sweep after the
                # (qb-1, hp=1) normalize chain (~6us of DVE+GPSIMD): the Tile
                # scheduler hoists injected work ~7 units earlier than issue
                # position, so the margin must be structural, not positional
                if hp == 1 and kb in (2, 4, 6, 8) and qb >= 1:
                    oproj_tile((qb - 1) * 4 + (kb - 2) // 2)
                if hp == 1 and kb in (10, 12) and qb + 1 < NG:
                    kq_group(
                        opp, qb + 1, w_q_sb, bq_sb, qT,
                        on_act=False, tag="o", js=(0 if kb == 10 else 1,),
                    )

    nc.compile()
    return nc


_NC_CACHE = None
_LAST_RESULT = None


def kernel(x, ln_scale, ln_bias, w_qkv, w_out):
    global _NC_CACHE, _LAST_RESULT
    if _NC_CACHE is None:
        _NC_CACHE = build_kernel()
    nc = _NC_CACHE

    import ml_dtypes

    x = np.asarray(x, np.float32)
    w_eff = (np.asarray(ln_scale, np.float32)[:, None] * np.asarray(w_qkv, np.float32))
    b_row = np.asarray(ln_bias, np.float32) @ np.asarray(w_qkv, np.float32)
    w_eff = w_eff.astype(ml_dtypes.bfloat16)
    w_out = np.asarray(w_out, np.float32).astype(ml_dtypes.bfloat16)

    in_maps = []
    for c in range(8):
        b, g = c // 2, c % 2
        s = slice(FPC * g, FPC * g + FPC)
        ks = slice(512 + FPC * g, 512 + FPC * g + FPC)
        vs = slice(1024 + FPC * g, 1024 + FPC * g + FPC)
        in_maps.append(
            {
                "xb": np.ascontiguousarray(x[b]),
                "wq": np.ascontiguousarray(w_eff[:, s]),
                "wk": np.ascontiguousarray(w_eff[:, ks]),
                "wv": np.ascontiguousarray(w_eff[:, vs]),
                "wo": np.ascontiguousarray(w_out[s, :]),
                "bq": np.ascontiguousarray(b_row[s]),
                "bk": np.ascontiguousarray(b_row[ks]),
                "bv": np.ascontiguousarray(b_row[vs]),
            }
        )
    res = run_bass_kernel_spmd(nc, in_maps, core_ids=list(range(8)))
    _LAST_RESULT = res
    outs = [res.results[c]["out"] for c in range(8)]
    return np.stack([outs[2 * b] + outs[2 * b + 1] for b in range(B)]).astype(
        np.float32
    )


if __name__ == "__main__":
    xs = np.random.randn(B, N, D).astype(np.float32)
    o = kernel(
        x=xs,
        ln_scale=np.ones(D, np.float32),
        ln_bias=np.zeros(D, np.float32),
        w_qkv=(np.random.randn(D, 3 * H * DH) / np.sqrt(D)).astype(np.float32),
        w_out=(np.random.randn(H * DH, D) / np.sqrt(H * DH)).astype(np.float32),
    )
    print(o.shape, o.dtype)
